# revision 4
# baseline (speedup 1.0000x reference)
"""ColAttention TRN2 kernel: out = gamma * colattn(x) + x.

Sharding: width. Core k gets x[:, :, :, 16k:16(k+1)] (contiguous after host
slice), so every HBM DMA on device is contiguous. Per core: 8 batches x 16
width columns = 128 independent attention problems over h=128.

Per (b, w) column pipeline on device:
  QK proj (f32r matmuls, PSUM-accumulated over 4 c-chunks)
  V^T_w (h,c) = xbf_slice.T @ (gamma*Wv).T  (bf16 matmuls, strided lhsT)
  S(i,j) = Q_w.T K_w (f32r, k=64)
  exp + row-sums via ACT accum_out; attn = exp * (1/sums) -> bf16
  attn_T via PE transpose; AV: out(c,i) = V^T.T @ attn_T (bf16)
  final (DVE fused): out = (AV + gamma*bv) + x   [in-place into the x slab]
"""

import numpy as np
import ml_dtypes

import concourse.bass as bass
from concourse import bacc, mybir
from concourse.tile import TileContext
from concourse.bass_utils import run_bass_kernel_spmd

f32 = mybir.dt.float32
f32r = mybir.dt.float32r
bf16 = mybir.dt.bfloat16
AF = mybir.ActivationFunctionType
ALU = mybir.AluOpType

N_CORES = 8
B, C, H, W = 8, 512, 128, 128
WT = W // N_CORES          # 16 w-columns per core
DQ = 64
NCH = C // 128             # 4 c-chunks

TRACE = False              # set True from test.py for profiling
LAST_RESULTS = None


def _build(bv_is_zero: bool):
    nc = bacc.Bacc("TRN2", num_devices=N_CORES, debug=False)

    x_d = nc.dram_tensor("x", (B, C, H, WT), f32r, kind="ExternalInput")
    wqk_d = nc.dram_tensor("wqkT", (C, 128), f32r, kind="ExternalInput")
    bqk_d = nc.dram_tensor("bqk", (128, 1), f32, kind="ExternalInput")
    wv_d = nc.dram_tensor("wvT", (C, C), bf16, kind="ExternalInput")
    gbv_d = nc.dram_tensor("gbv", (128, NCH), f32, kind="ExternalInput")
    out_d = nc.dram_tensor("out", (B, C, H, WT), f32, kind="ExternalOutput")
    id_d = nc.inline_tensor(np.eye(128, dtype=ml_dtypes.bfloat16), name="id128")

    xa = x_d.ap()
    oa = out_d.ap()

    with TileContext(nc) as tc:
        with (
            tc.tile_pool(name="const", bufs=1) as cpool,
            tc.tile_pool(name="xs", bufs=2) as xspool,
            tc.tile_pool(name="xb", bufs=2) as xbpool,
            tc.tile_pool(name="qk", bufs=2) as qkpool,
            tc.tile_pool(name="small", bufs=5) as spool,
            tc.tile_pool(name="pvt", bufs=2, space="PSUM") as pvt,
            tc.tile_pool(name="pshared", bufs=4, space="PSUM") as psh,
            tc.tile_pool(name="pav", bufs=2, space="PSUM") as pav,
        ):
            # ---- constants ----
            wqk_sb = cpool.tile([128, 128 * NCH], f32r, name="wqk_sb")
            for ci in range(NCH):
                nc.sync.dma_start(wqk_sb[:, ci * 128:(ci + 1) * 128],
                                  wqk_d.ap()[ci * 128:(ci + 1) * 128, :])
            wv_sb = cpool.tile([128, 512 * NCH], bf16, name="wv_sb")
            for ci in range(NCH):
                nc.sync.dma_start(wv_sb[:, ci * 512:(ci + 1) * 512],
                                  wv_d.ap()[ci * 128:(ci + 1) * 128, :])
            bqk_sb = cpool.tile([128, 1], f32, name="bqk_sb")
            nc.sync.dma_start(bqk_sb[:], bqk_d.ap())
            gbv_sb = cpool.tile([128, NCH], f32, name="gbv_sb")
            nc.sync.dma_start(gbv_sb[:], gbv_d.ap())
            id_sb = cpool.tile([128, 128], bf16, name="id_sb")
            nc.sync.dma_start(id_sb[:], id_d.ap())

            for b in range(B):
                # ---- batch prologue: hoisted into previous batch's w-loop ----
                with tc.high_priority(offset=0 if b == 0 else 200):
                    # load slab (4 chunks, contiguous 1 MiB each)
                    xs = xspool.tile([128, NCH * H * WT], f32r, tag="xs", name=f"xs{b}")
                    xs4 = xs[:].rearrange("p (c h w) -> p c h w", c=NCH, w=WT)
                    for ci in range(NCH):
                        nc.sync.dma_start(xs4[:, ci], xa[b, ci * 128:(ci + 1) * 128])

                    # bf16 copy of the slab (for V^T lhsT)
                    xb = xbpool.tile([128, NCH * H * WT], bf16, tag="xb", name=f"xb{b}")
                    for ci in range(NCH):
                        nc.any.tensor_copy(xb[:, ci * 2048:(ci + 1) * 2048],
                                           xs[:, ci * 2048:(ci + 1) * 2048])
                    xb4 = xb[:].rearrange("p (c h w) -> p c h w", c=NCH, w=WT)

                    # QK projection: full (h,w) range, n-tiles of 512
                    qk_sb = qkpool.tile([128, H * WT], f32r, tag="qk", name=f"qk{b}")
                    ks = qkpool.tile([64, H * WT], f32r, tag="ks", name=f"ks{b}")
                    for nt in range(H * WT // 512):
                        qkp = psh.tile([128, 512], f32, tag="ps1")
                        for ci in range(NCH):
                            nc.tensor.matmul(
                                qkp[:],
                                wqk_sb[:, ci * 128:(ci + 1) * 128],
                                xs[:, ci * 2048 + nt * 512: ci * 2048 + (nt + 1) * 512],
                                start=(ci == 0), stop=(ci == NCH - 1))
                        nc.scalar.activation(qk_sb[:, nt * 512:(nt + 1) * 512], qkp[:],
                                             AF.Identity, bias=bqk_sb[:])
                        # K rows 64:128 -> partitions 0:63 (scores needs same base)
                        nc.sync.dma_start(ks[:, nt * 512:(nt + 1) * 512],
                                          qk_sb[64:128, nt * 512:(nt + 1) * 512])
                qk3 = qk_sb[:].rearrange("p (h w) -> p h w", w=WT)
                ks3 = ks[:].rearrange("p (h w) -> p h w", w=WT)

                for w in range(WT):
                    # ---- V^T_w (h, c) ----
                    vt = pvt.tile([128, 512], f32, tag="vt")
                    for ci in range(NCH):
                        nc.tensor.matmul(vt[:], xb4[:, ci, :, w],
                                         wv_sb[:, ci * 512:(ci + 1) * 512],
                                         start=(ci == 0), stop=(ci == NCH - 1))
                    v_sb = spool.tile([128, 512], bf16, tag="v_sb")
                    nc.any.tensor_copy(v_sb[:], vt[:])

                    # ---- scores S(i,j), k=64 ----
                    sc = psh.tile([128, 128], f32, tag="ps1")
                    nc.tensor.matmul(sc[:], qk3[0:64, :, w], ks3[:, :, w],
                                     start=True, stop=True)

                    # ---- softmax (unnormalized exp + row sums) ----
                    ex = spool.tile([128, 128], f32, tag="ex")
                    sums = spool.tile([128, 1], f32, tag="sums")
                    nc.scalar.activation(ex[:], sc[:], AF.Exp, accum_out=sums[:])
                    rr = spool.tile([128, 1], f32, tag="rr")
                    nc.vector.reciprocal(rr[:], sums[:])
                    at = spool.tile([128, 128], bf16, tag="at")
                    nc.vector.tensor_scalar_mul(at[:], ex[:], rr[:])

                    # ---- attn^T via PE transpose ----
                    atp = psh.tile([128, 128], bf16, tag="ps1")
                    nc.tensor.transpose(atp[:], at[:], id_sb[:])
                    ats = spool.tile([128, 128], bf16, tag="ats")
                    nc.any.tensor_copy(ats[:], atp[:])

                    # ---- AV: out(c, i) per c-chunk into one bank ----
                    av = pav.tile([128, 512], f32, tag="av")
                    for ci in range(NCH):
                        nc.tensor.matmul(av[:, ci * 128:(ci + 1) * 128],
                                         v_sb[:, ci * 128:(ci + 1) * 128],
                                         ats[:], start=True, stop=True)

                    # ---- fused final: out = (AV + gamma*bv) + x, in-place ----
                    av3 = av[:].rearrange("p (c h) -> p c h", c=NCH)
                    if bv_is_zero:
                        nc.vector.scalar_tensor_tensor(
                            xs4[:, :, :, w], av3, 0.0, xs4[:, :, :, w],
                            ALU.add, ALU.add)
                    else:
                        for ci in range(NCH):
                            nc.vector.scalar_tensor_tensor(
                                xs4[:, ci, :, w], av3[:, ci],
                                gbv_sb[:, ci:ci + 1], xs4[:, ci, :, w],
                                ALU.add, ALU.add)

                # ---- store slab ----
                for ci in range(NCH):
                    nc.sync.dma_start(oa[b, ci * 128:(ci + 1) * 128],
                                      xs4[:, ci].bitcast(f32))

    nc.compile()
    return nc


def kernel(x, Wq, bq, Wk, bk, Wv, bv, gamma):
    global LAST_RESULTS
    x = np.ascontiguousarray(np.asarray(x, dtype=np.float32))
    Wq = np.asarray(Wq, dtype=np.float32)
    bq = np.asarray(bq, dtype=np.float32)
    Wk = np.asarray(Wk, dtype=np.float32)
    bk = np.asarray(bk, dtype=np.float32)
    Wv = np.asarray(Wv, dtype=np.float32)
    bv = np.asarray(bv, dtype=np.float32)
    g = float(np.asarray(gamma, dtype=np.float32).reshape(-1)[0])

    bv_is_zero = not np.any(bv)
    nc = _build(bv_is_zero)

    wqkT = np.ascontiguousarray(np.concatenate([Wq, Wk], axis=0).T)      # (C, 128)
    bqk = np.concatenate([bq, bk], axis=0).reshape(128, 1)
    wvT = np.ascontiguousarray((g * Wv).T).astype(ml_dtypes.bfloat16)    # (C, C)
    gbv = np.ascontiguousarray((g * bv).reshape(NCH, 128).T)             # (128, NCH)

    in_maps = []
    for k in range(N_CORES):
        in_maps.append({
            "x": np.ascontiguousarray(x[:, :, :, k * WT:(k + 1) * WT]),
            "wqkT": wqkT,
            "bqk": bqk,
            "wvT": wvT,
            "gbv": gbv,
        })

    res = run_bass_kernel_spmd(nc, in_maps, core_ids=list(range(N_CORES)),
                               trace=TRACE)
    LAST_RESULTS = res

    out = np.empty((B, C, H, W), dtype=np.float32)
    for k in range(N_CORES):
        out[:, :, :, k * WT:(k + 1) * WT] = res.results[k]["out"]
    return out


# revision 5
# speedup vs baseline: 1.5305x; 1.5305x over previous
"""ColAttention TRN2 kernel: out = gamma * colattn(x) + x.

Sharding: width. Core k gets x[:, :, :, 16k:16(k+1)] (contiguous after host
slice), so every HBM DMA on device is contiguous. Per core: 8 batches x 16
width columns = 128 independent attention problems over h=128.

Per (b, w) column pipeline on device:
  QK proj (f32r matmuls, PSUM-accumulated over 4 c-chunks)
  V^T_w (h,c) = xbf_slice.T @ (gamma*Wv).T  (bf16 matmuls, strided lhsT)
  S(i,j) = Q_w.T K_w (f32r, k=64)
  exp + row-sums via ACT accum_out; attn = exp * (1/sums) -> bf16
  attn_T via PE transpose; AV: out(c,i) = V^T.T @ attn_T (bf16)
  final (DVE fused): out = (AV + gamma*bv) + x   [in-place into the x slab]
"""

import numpy as np
import ml_dtypes

import concourse.bass as bass
from concourse import bacc, mybir
from concourse.tile import TileContext
from concourse.bass_utils import run_bass_kernel_spmd

f32 = mybir.dt.float32
f32r = mybir.dt.float32r
bf16 = mybir.dt.bfloat16
AF = mybir.ActivationFunctionType
ALU = mybir.AluOpType

N_CORES = 8
B, C, H, W = 8, 512, 128, 128
WT = W // N_CORES          # 16 w-columns per core
DQ = 64
NCH = C // 128             # 4 c-chunks

TRACE = False              # set True from test.py for profiling
LAST_RESULTS = None


def _build(bv_is_zero: bool):
    nc = bacc.Bacc("TRN2", num_devices=N_CORES, debug=False)

    x_d = nc.dram_tensor("x", (B, C, H, WT), f32r, kind="ExternalInput")
    wqk_d = nc.dram_tensor("wqkT", (C, 128), f32r, kind="ExternalInput")
    bqk_d = nc.dram_tensor("bqk", (128, 1), f32, kind="ExternalInput")
    wv_d = nc.dram_tensor("wvT", (C, C), bf16, kind="ExternalInput")
    gbv_d = nc.dram_tensor("gbv", (128, NCH), f32, kind="ExternalInput")
    out_d = nc.dram_tensor("out", (B, C, H, WT), f32, kind="ExternalOutput")
    id_d = nc.inline_tensor(np.eye(128, dtype=ml_dtypes.bfloat16), name="id128")

    xa = x_d.ap()
    oa = out_d.ap()

    with TileContext(nc) as tc:
        with (
            tc.tile_pool(name="const", bufs=1) as cpool,
            tc.tile_pool(name="xs", bufs=2) as xspool,
            tc.tile_pool(name="xb", bufs=2) as xbpool,
            tc.tile_pool(name="qk", bufs=2) as qkpool,
            tc.tile_pool(name="small", bufs=5) as spool,
            tc.tile_pool(name="pqk", bufs=1, space="PSUM") as pqk,
            tc.tile_pool(name="pvt", bufs=2, space="PSUM") as pvt,
            tc.tile_pool(name="psc", bufs=2, space="PSUM") as psc,
            tc.tile_pool(name="ptp", bufs=1, space="PSUM") as ptp,
            tc.tile_pool(name="pav", bufs=2, space="PSUM") as pav,
        ):
            # ---- constants ----
            wqk_sb = cpool.tile([128, 128 * NCH], f32r, name="wqk_sb")
            for ci in range(NCH):
                nc.sync.dma_start(wqk_sb[:, ci * 128:(ci + 1) * 128],
                                  wqk_d.ap()[ci * 128:(ci + 1) * 128, :])
            wv_sb = cpool.tile([128, 512 * NCH], bf16, name="wv_sb")
            for ci in range(NCH):
                nc.sync.dma_start(wv_sb[:, ci * 512:(ci + 1) * 512],
                                  wv_d.ap()[ci * 128:(ci + 1) * 128, :])
            bqk_sb = cpool.tile([128, 1], f32, name="bqk_sb")
            nc.sync.dma_start(bqk_sb[:], bqk_d.ap())
            gbv_sb = cpool.tile([128, NCH], f32, name="gbv_sb")
            nc.sync.dma_start(gbv_sb[:], gbv_d.ap())
            id_sb = cpool.tile([128, 128], bf16, name="id_sb")
            nc.sync.dma_start(id_sb[:], id_d.ap())

            for b in range(B):
                # ---- batch prologue: hoisted into previous batch's w-loop ----
                with tc.high_priority(offset=0 if b == 0 else 200):
                    # load slab (4 chunks, contiguous 1 MiB each)
                    xs = xspool.tile([128, NCH * H * WT], f32r, tag="xs", name=f"xs{b}")
                    xs4 = xs[:].rearrange("p (c h w) -> p c h w", c=NCH, w=WT)
                    for ci in range(NCH):
                        nc.sync.dma_start(xs4[:, ci], xa[b, ci * 128:(ci + 1) * 128])

                    # bf16 copy of the slab (for V^T lhsT)
                    xb = xbpool.tile([128, NCH * H * WT], bf16, tag="xb", name=f"xb{b}")
                    for ci in range(NCH):
                        if ci % 2 == 0:
                            nc.vector.tensor_copy(xb[:, ci * 2048:(ci + 1) * 2048],
                                                  xs[:, ci * 2048:(ci + 1) * 2048])
                        else:
                            nc.scalar.activation(xb[:, ci * 2048:(ci + 1) * 2048],
                                                 xs[:, ci * 2048:(ci + 1) * 2048],
                                                 AF.Identity)
                    xb4 = xb[:].rearrange("p (c h w) -> p c h w", c=NCH, w=WT)

                    # QK projection: full (h,w) range, n-tiles of 512
                    qk_sb = qkpool.tile([128, H * WT], f32r, tag="qk", name=f"qk{b}")
                    ks = qkpool.tile([64, H * WT], f32r, tag="ks", name=f"ks{b}")
                    for nt in range(H * WT // 512):
                        qkp = pqk.tile([128, 512], f32, tag="qkp")
                        for ci in range(NCH):
                            nc.tensor.matmul(
                                qkp[:],
                                wqk_sb[:, ci * 128:(ci + 1) * 128],
                                xs[:, ci * 2048 + nt * 512: ci * 2048 + (nt + 1) * 512],
                                start=(ci == 0), stop=(ci == NCH - 1))
                        nc.scalar.activation(qk_sb[:, nt * 512:(nt + 1) * 512], qkp[:],
                                             AF.Identity, bias=bqk_sb[:])
                        # K rows 64:128 -> partitions 0:63 (scores needs same base)
                        nc.sync.dma_start(ks[:, nt * 512:(nt + 1) * 512],
                                          qk_sb[64:128, nt * 512:(nt + 1) * 512])
                qk3 = qk_sb[:].rearrange("p (h w) -> p h w", w=WT)
                ks3 = ks[:].rearrange("p (h w) -> p h w", w=WT)

                for w in range(WT):
                    # ---- V^T_w (h, c) ----
                    vt = pvt.tile([128, 512], f32, tag="vt")
                    for ci in range(NCH):
                        nc.tensor.matmul(vt[:], xb4[:, ci, :, w],
                                         wv_sb[:, ci * 512:(ci + 1) * 512],
                                         start=(ci == 0), stop=(ci == NCH - 1))
                    v_sb = spool.tile([128, 512], bf16, tag="v_sb")
                    nc.scalar.activation(v_sb[:], vt[:], AF.Identity)

                    # ---- scores S(i,j), k=64 ----
                    sc = psc.tile([128, 128], f32, tag="sc")
                    nc.tensor.matmul(sc[:], qk3[0:64, :, w], ks3[:, :, w],
                                     start=True, stop=True)

                    # ---- softmax (unnormalized exp + row sums) ----
                    ex = spool.tile([128, 128], f32, tag="ex")
                    sums = spool.tile([128, 1], f32, tag="sums")
                    nc.scalar.activation(ex[:], sc[:], AF.Exp, accum_out=sums[:])
                    rr = spool.tile([128, 1], f32, tag="rr")
                    nc.vector.reciprocal(rr[:], sums[:])
                    at = spool.tile([128, 128], bf16, tag="at")
                    nc.vector.tensor_scalar_mul(at[:], ex[:], rr[:])

                    # ---- attn^T via PE transpose ----
                    atp = ptp.tile([128, 128], bf16, tag="atp")
                    nc.tensor.transpose(atp[:], at[:], id_sb[:])
                    ats = spool.tile([128, 128], bf16, tag="ats")
                    nc.vector.tensor_copy(ats[:], atp[:])

                    # ---- AV: out(c, i) per c-chunk into one bank ----
                    av = pav.tile([128, 512], f32, tag="av")
                    for ci in range(NCH):
                        nc.tensor.matmul(av[:, ci * 128:(ci + 1) * 128],
                                         v_sb[:, ci * 128:(ci + 1) * 128],
                                         ats[:], start=True, stop=True)

                    # ---- fused final: out = (AV + gamma*bv) + x, in-place ----
                    av3 = av[:].rearrange("p (c h) -> p c h", c=NCH)
                    if bv_is_zero:
                        nc.vector.scalar_tensor_tensor(
                            xs4[:, :, :, w], av3, 0.0, xs4[:, :, :, w],
                            ALU.add, ALU.add)
                    else:
                        for ci in range(NCH):
                            nc.vector.scalar_tensor_tensor(
                                xs4[:, ci, :, w], av3[:, ci],
                                gbv_sb[:, ci:ci + 1], xs4[:, ci, :, w],
                                ALU.add, ALU.add)

                # ---- store slab ----
                for ci in range(NCH):
                    nc.sync.dma_start(oa[b, ci * 128:(ci + 1) * 128],
                                      xs4[:, ci].bitcast(f32))

    nc.compile()
    return nc


def kernel(x, Wq, bq, Wk, bk, Wv, bv, gamma):
    global LAST_RESULTS
    x = np.ascontiguousarray(np.asarray(x, dtype=np.float32))
    Wq = np.asarray(Wq, dtype=np.float32)
    bq = np.asarray(bq, dtype=np.float32)
    Wk = np.asarray(Wk, dtype=np.float32)
    bk = np.asarray(bk, dtype=np.float32)
    Wv = np.asarray(Wv, dtype=np.float32)
    bv = np.asarray(bv, dtype=np.float32)
    g = float(np.asarray(gamma, dtype=np.float32).reshape(-1)[0])

    bv_is_zero = not np.any(bv)
    nc = _build(bv_is_zero)

    wqkT = np.ascontiguousarray(np.concatenate([Wq, Wk], axis=0).T)      # (C, 128)
    bqk = np.concatenate([bq, bk], axis=0).reshape(128, 1)
    wvT = np.ascontiguousarray((g * Wv).T).astype(ml_dtypes.bfloat16)    # (C, C)
    gbv = np.ascontiguousarray((g * bv).reshape(NCH, 128).T)             # (128, NCH)

    in_maps = []
    for k in range(N_CORES):
        in_maps.append({
            "x": np.ascontiguousarray(x[:, :, :, k * WT:(k + 1) * WT]),
            "wqkT": wqkT,
            "bqk": bqk,
            "wvT": wvT,
            "gbv": gbv,
        })

    res = run_bass_kernel_spmd(nc, in_maps, core_ids=list(range(N_CORES)),
                               trace=TRACE)
    LAST_RESULTS = res

    out = np.empty((B, C, H, W), dtype=np.float32)
    for k in range(N_CORES):
        out[:, :, :, k * WT:(k + 1) * WT] = res.results[k]["out"]
    return out
